# revision 1
# baseline (speedup 1.0000x reference)
"""MoE expert-gating kernel for 8 Trainium2 NeuronCores.

Problem (nn_ExpertGating): router MLP (H->H relu, H->E) + softmax + top-2
gating + weighted combine of per-expert outputs.

Sharding: data-parallel over the B*S=8192 tokens -> 1024 tokens per core.
Each core runs the full router for its tokens and combines its slice of all
8 experts' outputs.  No collectives needed; host concatenates the slices.

Per-core pipeline (T=1024 tokens, H=1024, E=8):
  1. x arrives host-pre-transposed, pre-split into fp16 hi + bf16 lo
     halves, AND pre-shuffled into the exact SBUF layout per 256-token
     segment, so every input DMA moves 4KB-contiguous partition lines
     (512B-run rearranges measured ~60GB/s vs ~250GB/s for 4KB runs).
  2. hT = relu(W1.T @ xT + b1) via 3 fp16/bf16 matmul passes (hi*hi;
     hi*lo + lo*hi), 1 cycle/row each.  This is the PE floor: the PE
     multiplies FP22 (~12-13 bit) operands, and both x and W1 need ~19
     bits for the logits to rank top-2 correctly (min top-2/3 margin on
     this data is ~5e-6; fp16x3 keeps logit error ~1e-6).  fp32 matmul is
     4 cycles/row (2 HW instructions) and f32r truncates both sides to
     ~12 bits (probed on HW), so neither beats 3x fp16.
  3. logitsT[e, t] += W2.T @ hT accumulated per m-block right after each
     relu (fp32: W2 also needs >13 bits), so segment logits are ready
     immediately after the segment's last stage-2 matmul.
  4. transpose logit chunks to [t, E] via PE (8x8 identity), softmax,
     top-2 via max8 + max_index, and launch the indirect-DMA gathers of
     each token's 2 selected expert rows.  expert_outputs are host-cast
     to fp16 (4 MB gathered instead of 8; rounding error ~2e-4 rel vs
     the 2e-2 budget): the SWDGE gather queue runs at only ~90-180GB/s
     and is the tail bottleneck.
  5. the weighted combine (out = g0*row0 + g1*row1) for segment s's
     chunks is DEFERRED TWO segments (emitted mid segment s+2's m-loop)
     and runs on the DVE: engine queues are strictly in-order, and a
     combine emitted earlier waits on its gather transfer,
     head-of-line-blocking the relu chain and stalling the PE.

DMA notes: every dma_start costs ~610ns of dispatch on the issuing queue,
so inputs are batched into a few transfers split across BOTH HWDGE rings
(SP: constants + xT segments; Activation: W1 k-quarters), each a separate
tile so tile-level dependencies stay fine-grained.  Segment 0 runs its
k-loop outermost, consuming W1 k-quarters as they land.
"""

import numpy as np

B, S, H, E = 4, 2048, 1024, 8
N_CORES = 8
T = (B * S) // N_CORES  # tokens per core
P = 128  # partitions
TCH = T // P  # token chunks per core (8)
KT = H // P  # contraction tiles (8)
HAL = 512  # psum pad width (full bank)
SEGS = [(0, 2), (2, 4), (4, 6), (6, 8)]
NSEG = len(SEGS)
SW = 2 * P  # segment width (tokens)
NQ = KT // 2  # w1 delivered in k-quarters
CBLOB = KT + KT * E + E + 1  # b1 | w2 | ident | b2  = 81 fp32 columns

_compiled_nc = None


def _build():
    import concourse.bacc as bacc
    import concourse.bass as bass
    import concourse.tile as tile
    from concourse import mybir

    f32 = mybir.dt.float32
    f16 = mybir.dt.float16
    bf16 = mybir.dt.bfloat16
    u32 = mybir.dt.uint32
    nc = bacc.Bacc("TRN2", target_bir_lowering=False, debug=False,
                   num_devices=N_CORES)

    # all inputs are host-shuffled to exact SBUF layout (4KB partition lines)
    xh = nc.dram_tensor("xh", [NSEG, P, KT, SW], f16, kind="ExternalInput").ap()
    xl = nc.dram_tensor("xl", [NSEG, P, KT, SW], bf16, kind="ExternalInput").ap()
    eo = nc.dram_tensor("eo", [E * T, H], f16, kind="ExternalInput").ap()
    w1h = nc.dram_tensor("w1h", [NQ, P, 2, H], f16, kind="ExternalInput").ap()
    w1l = nc.dram_tensor("w1l", [NQ, P, 2, H], bf16, kind="ExternalInput").ap()
    cblob = nc.dram_tensor("cblob", [P, CBLOB], f32, kind="ExternalInput").ap()
    iotad = nc.dram_tensor("iota", [P, 1], u32, kind="ExternalInput").ap()
    out = nc.dram_tensor("out", [T, H], f32, kind="ExternalOutput").ap()

    with tile.TileContext(nc) as tc:
        with (
            tc.tile_pool(name="singles", bufs=1) as singles,
            tc.tile_pool(name="eopool", bufs=6) as eopool,
            tc.tile_pool(name="accpool", bufs=3) as accpool,
            tc.tile_pool(name="smalls", bufs=8) as smalls,
            tc.tile_pool(name="ltpool", bufs=2) as ltpool,
            tc.tile_pool(name="psum", bufs=8, space="PSUM") as psum,
        ):
            # ---- input DMAs: SP ring = constants + xT, Activation ring =
            # W1 k-quarters (separate tiles per transfer -> fine deps) ----
            xh_s, xl_s, w1h_q, w1l_q = [], [], [], []
            for s in range(2):  # seg0, seg1 x before w1 (k-outer needs x first)
                th = singles.tile([P, KT, SW], f16, tag="xhs", name=f"xhs{s}",
                                  bufs=NSEG)
                nc.sync.dma_start(out=th[:], in_=xh[s])
                tl = singles.tile([P, KT, SW], bf16, tag="xls", name=f"xls{s}",
                                  bufs=NSEG)
                nc.sync.dma_start(out=tl[:], in_=xl[s])
                xh_s.append(th)
                xl_s.append(tl)
            # constants AFTER the seg0/1 x blobs: x gates the first matmul
            # (~13us) while b1/w2/ident/b2/iota aren't read before ~30us
            cb = singles.tile([P, CBLOB], f32)
            nc.sync.dma_start(out=cb[:], in_=cblob)
            iota_u = singles.tile([P, 1], u32)
            nc.sync.dma_start(out=iota_u[:], in_=iotad)
            b1_sb = cb[:, 0:KT]                       # b1_sb[p,m] = b1[m*128+p]
            w2col = lambda j: cb[:, KT + j * E:KT + (j + 1) * E]  # W2[j*128+p, e]
            ident = cb[0:E, KT + KT * E:KT + KT * E + E]          # eye(8)
            b2_sb = cb[0:E, CBLOB - 1:CBLOB]
            for q in range(NQ):
                th = singles.tile([P, 2, H], f16, tag="w1hq", name=f"w1hq{q}",
                                  bufs=NQ)
                nc.scalar.dma_start(out=th[:], in_=w1h[q])
                tl = singles.tile([P, 2, H], bf16, tag="w1lq", name=f"w1lq{q}",
                                  bufs=NQ)
                nc.scalar.dma_start(out=tl[:], in_=w1l[q])
                w1h_q.append(th)
                w1l_q.append(tl)
            for s in range(2, NSEG):
                th = singles.tile([P, KT, SW], f16, tag="xhs", name=f"xhs{s}",
                                  bufs=NSEG)
                nc.sync.dma_start(out=th[:], in_=xh[s])
                tl = singles.tile([P, KT, SW], bf16, tag="xls", name=f"xls{s}",
                                  bufs=NSEG)
                nc.sync.dma_start(out=tl[:], in_=xl[s])
                xh_s.append(th)
                xl_s.append(tl)

            hT = singles.tile([P, KT, T], f32)  # hT[p,m,t] = relu(x@W1+b1)[t, m*128+p]

            def mm3(ps, si, m, k, start, stop):
                msl = slice(m * P, (m + 1) * P)
                wh = w1h_q[k // 2][:, k % 2, msl]
                wl = w1l_q[k // 2][:, k % 2, msl]
                nc.tensor.matmul(ps[:], lhsT=wh, rhs=xh_s[si][:, k, :],
                                 start=start, stop=False)
                nc.tensor.matmul(ps[:], lhsT=wl, rhs=xh_s[si][:, k, :],
                                 start=False, stop=False)
                nc.tensor.matmul(ps[:], lhsT=wh, rhs=xl_s[si][:, k, :],
                                 start=False, stop=stop)

            def relu_stage3(ps, ps3, m, sl):
                nc.scalar.activation(
                    out=hT[:, m, sl], in_=ps[:],
                    func=mybir.ActivationFunctionType.Relu,
                    bias=b1_sb[:, m:m + 1], scale=1.0)
                nc.tensor.matmul(
                    ps3[:], lhsT=w2col(m), rhs=hT[:, m, sl],
                    start=(m == 0), stop=(m == KT - 1))

            # phase A: softmax, top-2, gather launch for one 128-token chunk
            def chunk_phase_a(lT, c0, tch):
                a = tch - c0
                pl = psum.tile([P, E], f32, tag="ps", name="pl",
                               padded_shape=[P, HAL])
                nc.tensor.transpose(pl[:], lT[:, a * P:(a + 1) * P], ident)
                negmax = smalls.tile([P, 1], f32, tag="negmax", name="negmax")
                nc.vector.reduce_max(negmax[:], pl[:],
                                     axis=mybir.AxisListType.X, negate=True)
                exps = smalls.tile([P, E], f32, tag="exps", name="exps")
                nc.scalar.activation(exps[:], pl[:],
                                     func=mybir.ActivationFunctionType.Exp,
                                     bias=negmax[:], scale=1.0)
                ssum = smalls.tile([P, 1], f32, tag="ssum", name="ssum")
                nc.vector.reduce_sum(ssum[:], exps[:], axis=mybir.AxisListType.X)
                rs = smalls.tile([P, 1], f32, tag="rs", name="rs")
                nc.vector.reciprocal(rs[:], ssum[:])
                # top-2 of exps == top-2 of probs; gate = exp * (1/sum)
                mx8 = smalls.tile([P, 8], f32, tag="mx8", name="mx8")
                nc.vector.max(mx8[:], exps[:])
                idx8 = smalls.tile([P, 8], u32, tag="idx8", name="idx8")
                nc.vector.max_index(idx8[:], mx8[:], exps[:])
                # flat eo row = expert*T + (tch*128 + partition)
                base = smalls.tile([P, 1], u32, tag="base", name="base")
                nc.vector.tensor_scalar_add(base[:], iota_u[:], tch * P)
                rows = smalls.tile([P, 2], u32, tag="rows", name="rows")
                for s in range(2):
                    nc.vector.tensor_scalar(
                        rows[:, s:s + 1], idx8[:, s:s + 1],
                        scalar1=T, scalar2=None, op0=mybir.AluOpType.mult)
                    nc.vector.tensor_tensor(
                        out=rows[:, s:s + 1], in0=rows[:, s:s + 1],
                        in1=base[:], op=mybir.AluOpType.add)
                eo_g = eopool.tile([P, 2, H], f16, tag="eog", name="eog")
                # NOTE: a single [P,2]-offset indirect DMA compiles and
                # simulates but dies at runtime (NRT INTERNAL) -- keep two
                # single-offset gathers
                for s in range(2):
                    nc.gpsimd.indirect_dma_start(
                        out=eo_g[:, s, :], out_offset=None, in_=eo,
                        in_offset=bass.IndirectOffsetOnAxis(
                            ap=rows[:, s:s + 1], axis=0))
                g0 = smalls.tile([P, 1], f32, tag="g0", name="g0")
                nc.vector.tensor_mul(g0[:], mx8[:, 0:1], rs[:])
                g1 = smalls.tile([P, 1], f32, tag="g1", name="g1")
                nc.vector.tensor_mul(g1[:], mx8[:, 1:2], rs[:])
                return (tch, eo_g, g0, g1)

            # phase B: weighted combine + output store (deferred).
            # Mid-kernel: both ops on DVE so the scalar relu chain never
            # waits on a gather.  Flush (tail): scalar ACTIVATE + DVE STT
            # so consecutive chunks pipeline across the two engines.
            def chunk_phase_b(st, flush=False):
                tch, eo_g, g0, g1 = st
                acc = accpool.tile([P, H], f32, tag="acc", name="acc")
                osl = slice(tch * P, (tch + 1) * P)
                if flush:
                    # tail: h-halves in parallel (scalar ACT || DVE mul),
                    # then each half's output DMA on its own HWDGE ring as
                    # soon as that half's accumulate lands
                    h0, h1 = slice(0, H // 2), slice(H // 2, H)
                    nc.scalar.activation(acc[:, h0], eo_g[:, 0, h0],
                                         func=mybir.ActivationFunctionType.Copy,
                                         scale=g0[:])
                    nc.vector.tensor_scalar_mul(acc[:, h1], eo_g[:, 0, h1],
                                                g0[:])
                    for half, ring in ((h0, nc.sync), (h1, nc.scalar)):
                        nc.vector.scalar_tensor_tensor(
                            out=acc[:, half], in0=eo_g[:, 1, half],
                            scalar=g1[:], in1=acc[:, half],
                            op0=mybir.AluOpType.mult, op1=mybir.AluOpType.add)
                        ring.dma_start(out=out[osl, half], in_=acc[:, half])
                else:
                    nc.vector.tensor_scalar_mul(acc[:], eo_g[:, 0, :], g0[:])
                    nc.vector.scalar_tensor_tensor(
                        out=acc[:], in0=eo_g[:, 1, :], scalar=g1[:], in1=acc[:],
                        op0=mybir.AluOpType.mult, op1=mybir.AluOpType.add)
                    nc.sync.dma_start(out=out[osl, :], in_=acc[:])

            pending = []
            for si, (c0, c1) in enumerate(SEGS):
                sl = slice(c0 * P, c1 * P)
                W = (c1 - c0) * P
                # ---- stage 2 (+ interleaved stage 3 and deferred combines) --
                if si == 0:
                    # k-outer: consume W1/xT k-blocks as the DMA delivers
                    # them.  ps3 must be allocated AFTER the 8 stage-2
                    # accumulators: the pool has exactly 8 slots and a 9th
                    # live tile ahead of them deadlocks the slot ring.
                    ps_m = [psum.tile([P, W], f32, tag="ps", name=f"ps{m}",
                                      padded_shape=[P, HAL]) for m in range(KT)]
                    for k in range(KT):
                        for m in range(KT):
                            mm3(ps_m[m], si, m, k, k == 0, k == KT - 1)
                    ps3 = psum.tile([E, W], f32, tag="ps", name="ps3",
                                    padded_shape=[E, HAL])
                    for m in range(KT):
                        relu_stage3(ps_m[m], ps3, m, sl)
                else:
                    ps3 = psum.tile([E, W], f32, tag="ps", name="ps3",
                                    padded_shape=[E, HAL])
                    for m in range(KT):
                        ps = psum.tile([P, W], f32, tag="ps", name="ps",
                                       padded_shape=[P, HAL])
                        for k in range(KT):
                            mm3(ps, si, m, k, k == 0, k == KT - 1)
                        relu_stage3(ps, ps3, m, sl)
                        # run a deferred combine once its gather is ~2
                        # segments old (keep >=2 chunks in flight)
                        if m in (3, 5, 7) and len(pending) > 2:
                            chunk_phase_b(pending.pop(0))

                lT = ltpool.tile([E, W], f32, tag="lT", name="lT",
                                 padded_shape=[E, HAL])
                nc.scalar.activation(out=lT[:], in_=ps3[:],
                                     func=mybir.ActivationFunctionType.Identity,
                                     bias=b2_sb, scale=1.0)
                # last segment: launch the final chunk's gathers first so
                # the tail's combine chain starts on the critical chunk
                order = range(c0, c1) if si < len(SEGS) - 1 else reversed(range(c0, c1))
                for tch in order:
                    pending.append(chunk_phase_a(lT, c0, tch))
            while pending:
                chunk_phase_b(pending.pop(0), flush=True)

    nc.compile()
    return nc


def _get_nc():
    global _compiled_nc
    if _compiled_nc is None:
        _compiled_nc = _build()
    return _compiled_nc


def _split_hi_lo(a):
    """fp16 hi + bf16 lo split of an fp32 array (lo unscaled; bf16's
    exponent range covers it)."""
    import ml_dtypes
    a = np.asarray(a, dtype=np.float32)
    hi = a.astype(np.float16)
    lo = (a.astype(np.float64) - hi.astype(np.float64)).astype(ml_dtypes.bfloat16)
    return hi, lo


def make_in_maps(hidden_states, expert_outputs, W1, b1, W2, b2):
    hs = np.ascontiguousarray(np.asarray(hidden_states, dtype=np.float32)).reshape(B * S, H)
    eo = np.asarray(expert_outputs, dtype=np.float32).reshape(E, B * S, H)
    w1hi, w1lo = _split_hi_lo(W1)
    # w1 k-quarter blobs in SBUF layout: [q, p, kk, m] = W1[(2q+kk)*128+p, m]
    w1hq = np.ascontiguousarray(
        w1hi.reshape(NQ, 2, P, H).transpose(0, 2, 1, 3))
    w1lq = np.ascontiguousarray(
        w1lo.reshape(NQ, 2, P, H).transpose(0, 2, 1, 3))
    b1v = np.asarray(b1, dtype=np.float32)
    w2 = np.asarray(W2, dtype=np.float32)
    b2v = np.asarray(b2, dtype=np.float32)
    # constants blob: b1 | w2 | ident | b2, all in on-chip layout
    cb = np.zeros((P, CBLOB), dtype=np.float32)
    cb[:, 0:KT] = b1v.reshape(KT, P).T                    # b1[m*128+p]
    cb[:, KT:KT + KT * E] = w2.reshape(KT, P, E).transpose(1, 0, 2).reshape(P, KT * E)
    cb[0:E, KT + KT * E:KT + KT * E + E] = np.eye(E, dtype=np.float32)
    cb[0:E, CBLOB - 1] = b2v
    iotav = np.arange(P, dtype=np.uint32).reshape(P, 1)
    in_maps = []
    for c in range(N_CORES):
        sl = slice(c * T, (c + 1) * T)
        xhi, xlo = _split_hi_lo(hs[sl].T)  # [H, T]
        # per-seg SBUF-layout blobs: [s, p, k, u] = xT[k*128+p, s*SW+u]
        xhb = np.ascontiguousarray(
            xhi.reshape(KT, P, NSEG, SW).transpose(2, 1, 0, 3))
        xlb = np.ascontiguousarray(
            xlo.reshape(KT, P, NSEG, SW).transpose(2, 1, 0, 3))
        in_maps.append({
            "xh": xhb, "xl": xlb,
            "eo": np.ascontiguousarray(
                eo[:, sl, :].reshape(E * T, H).astype(np.float16)),
            "w1h": w1hq, "w1l": w1lq, "cblob": cb, "iota": iotav,
        })
    return in_maps


def kernel(hidden_states, expert_outputs, W1, b1, W2, b2, k=2):
    from concourse.bass_utils import run_bass_kernel_spmd

    in_maps = make_in_maps(hidden_states, expert_outputs, W1, b1, W2, b2)
    nc = _get_nc()
    res = run_bass_kernel_spmd(nc, in_maps, core_ids=list(range(N_CORES)))
    full = np.concatenate([res.results[c]["out"] for c in range(N_CORES)], axis=0)
    return full.reshape(B, S, H)



# revision 7
# speedup vs baseline: 1.0906x; 1.0906x over previous
"""MoE expert-gating kernel for 8 Trainium2 NeuronCores.

Problem (nn_ExpertGating): router MLP (H->H relu, H->E) + softmax + top-2
gating + weighted combine of per-expert outputs.

Sharding: data-parallel over the B*S=8192 tokens -> 1024 tokens per core.
Each core runs the full router for its tokens and combines its slice of all
8 experts' outputs.  No collectives needed; host concatenates the slices.

v2 changes over the 145us baseline (which idled the PE ~45% of the span):
  * Inputs arrive as fused u16 blobs (fp16 hi | bf16 lo interleaved per
    k-slice) and are bitcast to f16/bf16 SBUF views -- halves the number
    of dma_start dispatches (~610ns each on the issuing engine queue).
  * Criticality-ordered delivery: seg0's x comes as four k-pair tiles and
    W1's first two k-slices as their own tiles, so the first matmul can
    start ~2us in instead of 12.5us.
  * seg0 runs k-outer over two m-halves (4 PSUM accumulators each);
    segments 1-3 run m-outer.
  * Software-pipelined PE stream: the stage-3 (W2) matmul for m-block j
    and the logit transposes for segment s are emitted 1-2 stage-2 blocks
    later, so the PE never sits behind the scalar engine's relu (the
    baseline lost ~430ns on every one of 32 m-blocks to that).
  * Combines pop earlier (pending<=2 steady state) and the output is
    written f16 (host upcasts), shrinking the post-PE tail from ~17us to
    ~6us.

Per-core pipeline (T=1024 tokens, H=1024, E=8): see mm3/relu/stage3 --
fp16x3 passes (hi*hi, lo_w*hi_x, hi_w*lo_x) are required because the
min top-2/3 prob margin on this data is ~5e-6; fewer passes misrank
tokens, and a single misranked token blows the absmax budget.
"""

import numpy as np

B, S, H, E = 4, 2048, 1024, 8
N_CORES = 8
T = (B * S) // N_CORES  # tokens per core
P = 128  # partitions
TCH = T // P  # token chunks per core (8)
KT = H // P  # contraction tiles (8)
HAL = 512  # psum pad width (full bank)
SEGS = [(0, 2), (2, 4), (4, 6), (6, 8)]
NSEG = len(SEGS)
SW = 2 * P  # segment width (tokens)
CBLOB = KT + KT * E + E + 1 + 1  # b1 | w2 | ident | b2 | iota-bits = 82 cols

_compiled_nc = None


def _build():
    import concourse.bacc as bacc
    import concourse.bass as bass
    import concourse.tile as tile
    from concourse import mybir

    f32 = mybir.dt.float32
    f16 = mybir.dt.float16
    bf16 = mybir.dt.bfloat16
    u16 = mybir.dt.uint16
    u32 = mybir.dt.uint32
    nc = bacc.Bacc("TRN2", target_bir_lowering=False, debug=False,
                   num_devices=N_CORES)

    # x: [p, k, hl, u] u16; seg0 additionally pair-split for early start
    xs0 = nc.dram_tensor("xs0", [KT // 2, P, 2, 2, SW], u16,
                         kind="ExternalInput").ap()
    xs = nc.dram_tensor("xs", [NSEG - 1, P, KT, 2, SW], u16,
                        kind="ExternalInput").ap()
    # w1: k0 and k1 solo, then k-pairs: [p, (kk,) hl, m] u16
    w1k0 = nc.dram_tensor("w1k0", [P, 2, H], u16, kind="ExternalInput").ap()
    w1k1 = nc.dram_tensor("w1k1", [P, 2, H], u16, kind="ExternalInput").ap()
    w1p = nc.dram_tensor("w1p", [3, P, 2, 2, H], u16, kind="ExternalInput").ap()
    eo = nc.dram_tensor("eo", [E * T, H], f16, kind="ExternalInput").ap()
    cblob = nc.dram_tensor("cblob", [P, CBLOB], f32, kind="ExternalInput").ap()
    out = nc.dram_tensor("out", [T, H], f16, kind="ExternalOutput").ap()

    with tile.TileContext(nc) as tc:
        with (
            tc.tile_pool(name="singles", bufs=1) as singles,
            tc.tile_pool(name="eopool", bufs=6) as eopool,
            tc.tile_pool(name="accpool", bufs=4) as accpool,
            tc.tile_pool(name="smalls", bufs=8) as smalls,
            tc.tile_pool(name="ltpool", bufs=2) as ltpool,
            tc.tile_pool(name="psum", bufs=8, space="PSUM") as psum,
        ):
            # ---- input DMAs, criticality-ordered ----
            # SP (sync) ring: x seg0 k-pairs, then seg1-3 blobs
            # ACT (scalar) ring: w1 k0, k1, pairs, then cblob
            x0_t = []
            for pr in range(2):  # first two k-pairs of seg0 before anything
                t = singles.tile([P, 2, 2, SW], u16, tag="x0", name=f"x0_{pr}",
                                 bufs=KT // 2)
                nc.sync.dma_start(out=t[:], in_=xs0[pr])
                x0_t.append(t)
            w1k0_t = singles.tile([P, 2, H], u16)
            nc.scalar.dma_start(out=w1k0_t[:], in_=w1k0)
            w1k1_t = singles.tile([P, 2, H], u16)
            nc.scalar.dma_start(out=w1k1_t[:], in_=w1k1)
            for pr in range(2, KT // 2):
                t = singles.tile([P, 2, 2, SW], u16, tag="x0", name=f"x0_{pr}",
                                 bufs=KT // 2)
                nc.sync.dma_start(out=t[:], in_=xs0[pr])
                x0_t.append(t)
            w1p_t = []
            for q in range(3):
                t = singles.tile([P, 2, 2, H], u16, tag="w1p", name=f"w1p{q}",
                                 bufs=3)
                nc.scalar.dma_start(out=t[:], in_=w1p[q])
                w1p_t.append(t)
            cb = singles.tile([P, CBLOB], f32)
            nc.scalar.dma_start(out=cb[:], in_=cblob)
            xs_t = []
            for s in range(NSEG - 1):
                t = singles.tile([P, KT, 2, SW], u16, tag="xs", name=f"xs{s}",
                                 bufs=NSEG - 1)
                nc.sync.dma_start(out=t[:], in_=xs[s])
                xs_t.append(t)

            b1_sb = cb[:, 0:KT]                       # b1_sb[p,m] = b1[m*128+p]
            w2col = lambda j: cb[:, KT + j * E:KT + (j + 1) * E]  # W2[j*128+p, e]
            ident = cb[0:E, KT + KT * E:KT + KT * E + E]          # eye(8)
            b2_sb = cb[0:E, KT + KT * E + E:KT + KT * E + E + 1]
            iota_u = cb[:, CBLOB - 1:CBLOB].bitcast(u32)

            def xsl(si, k, hl):
                dt = f16 if hl == 0 else bf16
                if si == 0:
                    return x0_t[k // 2][:, k % 2, hl, :].bitcast(dt)
                return xs_t[si - 1][:, k, hl, :].bitcast(dt)

            def wsl(k, hl, msl):
                dt = f16 if hl == 0 else bf16
                if k == 0:
                    return w1k0_t[:, hl, msl].bitcast(dt)
                if k == 1:
                    return w1k1_t[:, hl, msl].bitcast(dt)
                return w1p_t[(k - 2) // 2][:, (k - 2) % 2, hl, msl].bitcast(dt)

            hT = singles.tile([P, KT, T], f32)  # hT[p,m,t] = relu(x@W1+b1)[t, m*128+p]

            def mm3(ps, si, m, k, start, stop):
                msl = slice(m * P, (m + 1) * P)
                wh, wl = wsl(k, 0, msl), wsl(k, 1, msl)
                nc.tensor.matmul(ps[:], lhsT=wh, rhs=xsl(si, k, 0),
                                 start=start, stop=False)
                nc.tensor.matmul(ps[:], lhsT=wl, rhs=xsl(si, k, 0),
                                 start=False, stop=False)
                nc.tensor.matmul(ps[:], lhsT=wh, rhs=xsl(si, k, 1),
                                 start=False, stop=stop)

            # ---- deferred PE work: stage3 + logit transpose/phase_a ----
            # s3q holds stage-3 matmuls whose relu has already been emitted
            # (stamped with the stage-2 block counter; popped only with a
            # >=1-block lag so the PE never waits on the scalar engine's
            # relu); paq holds per-chunk transpose+phase_a work.  One slot
            # after every stage-2 block drains one paq entry (gathers are
            # latency-critical) plus up to two stage3 entries.
            s3q = []
            paq = []
            pending = []
            blk = {"n": 0}

            def emit_relu(ps, ps3, si, m):
                sl = slice(SEGS[si][0] * P, SEGS[si][1] * P)
                nc.scalar.activation(
                    out=hT[:, m, sl], in_=ps[:],
                    func=mybir.ActivationFunctionType.Relu,
                    bias=b1_sb[:, m:m + 1], scale=1.0)
                s3q.append((blk["n"], ps3, m, sl, si))

            def emit_stage3(ent):
                _, ps3, m, sl, si = ent
                nc.tensor.matmul(
                    ps3[:], lhsT=w2col(m), rhs=hT[:, m, sl],
                    start=(m == 0), stop=(m == KT - 1))
                if m == KT - 1:
                    lT = ltpool.tile([E, SEGS[si][1] * P - SEGS[si][0] * P],
                                     f32, tag="lT", name="lT",
                                     padded_shape=[E, HAL])
                    nc.scalar.activation(
                        out=lT[:], in_=ps3[:],
                        func=mybir.ActivationFunctionType.Identity,
                        bias=b2_sb, scale=1.0)
                    c0, c1 = SEGS[si]
                    order = (range(c0, c1) if si < NSEG - 1
                             else reversed(range(c0, c1)))
                    for tch in order:
                        paq.append((lT, c0, tch))

            # phase A: transpose+softmax+top-2+gather launch for a 128-chunk
            def chunk_phase_a(lT, c0, tch):
                a = tch - c0
                pl = psum.tile([P, E], f32, tag="pl", name="pl", bufs=1,
                               padded_shape=[P, HAL])
                nc.tensor.transpose(pl[:], lT[:, a * P:(a + 1) * P], ident)
                negmax = smalls.tile([P, 1], f32, tag="negmax", name="negmax")
                nc.vector.reduce_max(negmax[:], pl[:],
                                     axis=mybir.AxisListType.X, negate=True)
                exps = smalls.tile([P, E], f32, tag="exps", name="exps")
                nc.scalar.activation(exps[:], pl[:],
                                     func=mybir.ActivationFunctionType.Exp,
                                     bias=negmax[:], scale=1.0)
                ssum = smalls.tile([P, 1], f32, tag="ssum", name="ssum")
                nc.vector.reduce_sum(ssum[:], exps[:], axis=mybir.AxisListType.X)
                rs = smalls.tile([P, 1], f32, tag="rs", name="rs")
                nc.vector.reciprocal(rs[:], ssum[:])
                # top-2 of exps == top-2 of probs; gate = exp * (1/sum)
                mx8 = smalls.tile([P, 8], f32, tag="mx8", name="mx8")
                nc.vector.max(mx8[:], exps[:])
                idx8 = smalls.tile([P, 8], u32, tag="idx8", name="idx8")
                nc.vector.max_index(idx8[:], mx8[:], exps[:])
                # flat eo row = expert*T + (tch*128 + partition)
                base = smalls.tile([P, 1], u32, tag="base", name="base")
                nc.vector.tensor_scalar_add(base[:], iota_u, tch * P)
                rows = smalls.tile([P, 2], u32, tag="rows", name="rows")
                for s in range(2):
                    nc.vector.tensor_scalar(
                        rows[:, s:s + 1], idx8[:, s:s + 1],
                        scalar1=T, scalar2=None, op0=mybir.AluOpType.mult)
                    nc.vector.tensor_tensor(
                        out=rows[:, s:s + 1], in0=rows[:, s:s + 1],
                        in1=base[:], op=mybir.AluOpType.add)
                eo_g = eopool.tile([P, 2, H], f16, tag="eog", name="eog")
                # NOTE: a single [P,2]-offset indirect DMA compiles and
                # simulates but dies at runtime (NRT INTERNAL) -- keep two
                # single-offset gathers
                for s in range(2):
                    nc.gpsimd.indirect_dma_start(
                        out=eo_g[:, s, :], out_offset=None, in_=eo,
                        in_offset=bass.IndirectOffsetOnAxis(
                            ap=rows[:, s:s + 1], axis=0))
                g0 = smalls.tile([P, 1], f32, tag="g0", name="g0")
                nc.vector.tensor_mul(g0[:], mx8[:, 0:1], rs[:])
                g1 = smalls.tile([P, 1], f32, tag="g1", name="g1")
                nc.vector.tensor_mul(g1[:], mx8[:, 1:2], rs[:])
                pending.append((tch, eo_g, g0, g1))

            # phase B: weighted combine + f16 output store
            def chunk_phase_b(st, flush=False):
                tch, eo_g, g0, g1 = st
                acc = accpool.tile([P, H], f16, tag="acc", name="acc")
                osl = slice(tch * P, (tch + 1) * P)
                if flush:
                    # tail: h-halves in parallel (scalar ACT || DVE mul),
                    # each half's output DMA as soon as it lands
                    h0, h1 = slice(0, H // 2), slice(H // 2, H)
                    nc.scalar.activation(acc[:, h0], eo_g[:, 0, h0],
                                         func=mybir.ActivationFunctionType.Copy,
                                         scale=g0[:])
                    nc.vector.tensor_scalar_mul(acc[:, h1], eo_g[:, 0, h1],
                                                g0[:])
                    for half, ring in ((h0, nc.sync), (h1, nc.scalar)):
                        nc.vector.scalar_tensor_tensor(
                            out=acc[:, half], in0=eo_g[:, 1, half],
                            scalar=g1[:], in1=acc[:, half],
                            op0=mybir.AluOpType.mult, op1=mybir.AluOpType.add)
                        ring.dma_start(out=out[osl, half], in_=acc[:, half])
                else:
                    nc.vector.tensor_scalar_mul(acc[:], eo_g[:, 0, :], g0[:])
                    nc.vector.scalar_tensor_tensor(
                        out=acc[:], in0=eo_g[:, 1, :], scalar=g1[:], in1=acc[:],
                        op0=mybir.AluOpType.mult, op1=mybir.AluOpType.add)
                    nc.sync.dma_start(out=out[osl, :], in_=acc[:])

            def drain_slot(allow_combine=True):
                # after each stage-2 block: one transpose+phase_a (gathers
                # first -- latency-critical), up to two lagged stage3s, and
                # possibly one combine of an old chunk
                if paq:
                    lT, c0, tch = paq.pop(0)
                    chunk_phase_a(lT, c0, tch)
                did = 0
                while s3q and did < 2 and s3q[0][0] < blk["n"]:
                    emit_stage3(s3q.pop(0))
                    did += 1
                if allow_combine and len(pending) >= 3:
                    chunk_phase_b(pending.pop(0))

            # ---- segment 0: k-outer over two m-halves (x and W1 k-slices
            # are consumed as the DMA delivers them) ----
            ps3_seg0 = None
            for half in range(2):
                ms = range(half * 4, half * 4 + 4)
                ps_m = {m: psum.tile([P, SW], f32, tag="ps2", name=f"ps{m}",
                                     bufs=5, padded_shape=[P, HAL])
                        for m in ms}
                for k in range(KT):
                    for m in ms:
                        mm3(ps_m[m], 0, m, k, k == 0, k == KT - 1)
                    if half == 1:
                        drain_slot()
                        blk["n"] += 1
                if half == 0:
                    ps3_seg0 = psum.tile([E, SW], f32, tag="ps3", name="ps3",
                                         bufs=2, padded_shape=[E, HAL])
                for m in ms:
                    emit_relu(ps_m[m], ps3_seg0, 0, m)
                blk["n"] += 1

            # ---- segments 1-3: m-outer, pipelined drains ----
            for si in range(1, NSEG):
                ps3 = psum.tile([E, SW], f32, tag="ps3", name="ps3",
                                bufs=2, padded_shape=[E, HAL])
                for m in range(KT):
                    ps = psum.tile([P, SW], f32, tag="ps2", name="ps",
                                   bufs=5, padded_shape=[P, HAL])
                    for k in range(KT):
                        mm3(ps, si, m, k, k == 0, k == KT - 1)
                    drain_slot()
                    emit_relu(ps, ps3, si, m)
                    blk["n"] += 1

            # ---- tail: drain everything left, then flush combines ----
            blk["n"] += 1
            while s3q or paq:
                drain_slot(allow_combine=False)
            while pending:
                chunk_phase_b(pending.pop(0), flush=True)

    nc.compile()
    return nc


def _get_nc():
    global _compiled_nc
    if _compiled_nc is None:
        _compiled_nc = _build()
    return _compiled_nc


def _split_hi_lo(a):
    """fp16 hi + bf16 lo split of an fp32 array (lo unscaled; bf16's
    exponent range covers it)."""
    import ml_dtypes
    a = np.asarray(a, dtype=np.float32)
    hi = a.astype(np.float16)
    lo = (a.astype(np.float64) - hi.astype(np.float64)).astype(ml_dtypes.bfloat16)
    return hi, lo


def make_in_maps(hidden_states, expert_outputs, W1, b1, W2, b2):
    hs = np.ascontiguousarray(np.asarray(hidden_states, dtype=np.float32)).reshape(B * S, H)
    eo = np.asarray(expert_outputs, dtype=np.float32).reshape(E, B * S, H)
    w1hi, w1lo = _split_hi_lo(W1)
    # w1 blob [k, p, hl, m] u16 = bits of W1[(k*128+p), m] hi/lo
    w1u = np.empty((KT, P, 2, H), dtype=np.uint16)
    w1u[:, :, 0, :] = w1hi.reshape(KT, P, H).view(np.uint16)
    w1u[:, :, 1, :] = w1lo.reshape(KT, P, H).view(np.uint16)
    w1k0 = np.ascontiguousarray(w1u[0])
    w1k1 = np.ascontiguousarray(w1u[1])
    # pairs [q, p, kk, hl, m]
    w1p = np.ascontiguousarray(w1u[2:].reshape(3, 2, P, 2, H).transpose(0, 2, 1, 3, 4))
    b1v = np.asarray(b1, dtype=np.float32)
    w2 = np.asarray(W2, dtype=np.float32)
    b2v = np.asarray(b2, dtype=np.float32)
    # constants blob: b1 | w2 | ident | b2 | iota-bits
    cblk = np.zeros((P, CBLOB), dtype=np.float32)
    cblk[:, 0:KT] = b1v.reshape(KT, P).T                    # b1[m*128+p]
    cblk[:, KT:KT + KT * E] = w2.reshape(KT, P, E).transpose(1, 0, 2).reshape(P, KT * E)
    cblk[0:E, KT + KT * E:KT + KT * E + E] = np.eye(E, dtype=np.float32)
    cblk[0:E, KT + KT * E + E] = b2v
    cblk[:, CBLOB - 1] = np.arange(P, dtype=np.uint32).view(np.float32)
    in_maps = []
    for c in range(N_CORES):
        sl = slice(c * T, (c + 1) * T)
        xhi, xlo = _split_hi_lo(hs[sl].T)  # [H, T]
        # x blob [s, p, k, hl, u] u16 = bits of xT[k*128+p, s*SW+u]
        xu = np.empty((NSEG, P, KT, 2, SW), dtype=np.uint16)
        xu[:, :, :, 0, :] = (xhi.reshape(KT, P, NSEG, SW)
                             .transpose(2, 1, 0, 3).view(np.uint16))
        xu[:, :, :, 1, :] = (xlo.reshape(KT, P, NSEG, SW)
                             .transpose(2, 1, 0, 3).view(np.uint16))
        # seg0 pair-major: [pr, p, kk, hl, u]
        xs0 = np.ascontiguousarray(
            xu[0].reshape(P, KT // 2, 2, 2, SW).transpose(1, 0, 2, 3, 4))
        xsr = np.ascontiguousarray(xu[1:])
        in_maps.append({
            "xs0": xs0, "xs": xsr,
            "eo": np.ascontiguousarray(
                eo[:, sl, :].reshape(E * T, H).astype(np.float16)),
            "w1k0": w1k0, "w1k1": w1k1, "w1p": w1p, "cblob": cblk,
        })
    return in_maps


def kernel(hidden_states, expert_outputs, W1, b1, W2, b2, k=2):
    from concourse.bass_utils import run_bass_kernel_spmd

    in_maps = make_in_maps(hidden_states, expert_outputs, W1, b1, W2, b2)
    nc = _get_nc()
    res = run_bass_kernel_spmd(nc, in_maps, core_ids=list(range(N_CORES)))
    full = np.concatenate([res.results[c]["out"].astype(np.float32)
                           for c in range(N_CORES)], axis=0)
    return full.reshape(B, S, H)


# revision 8
# speedup vs baseline: 1.1474x; 1.0521x over previous
"""MoE expert-gating kernel for 8 Trainium2 NeuronCores.

Problem (nn_ExpertGating): router MLP (H->H relu, H->E) + softmax + top-2
gating + weighted combine of per-expert outputs.

Sharding: data-parallel over the B*S=8192 tokens -> 1024 tokens per core.
Each core runs the full router for its tokens and combines its slice of all
8 experts' outputs.  No collectives needed; host concatenates the slices.

v3 structure (baseline was 152us, v2 140us):
  * Inputs arrive as fused u16 blobs (fp16 hi | bf16 lo interleaved per
    k-slice) bitcast to f16/bf16 SBUF views -- halves dma_start count
    (~610ns dispatch each on the issuing engine queue).
  * Criticality-ordered, ring-balanced delivery: sync ring carries seg0's
    four x k-pair tiles then xs1/xs3/xs4; scalar ring carries W1 (k0, k1,
    then pairs), the constants blob, then xs2.  First matmul needs only
    x-pair0 + W1-k0 (~0.8MB), so the PE starts right after the ~7us
    framework preamble + DMA latency instead of waiting for 9MB.
  * seg0 runs k-outer over two m-halves (PSUM accumulators per half) so
    it can consume x/W1 k-slices as they land; later segments m-outer.
  * Stage-3 (W2, fp32) matmuls are batched per segment into one burst in
    the next segment's m=0 slot: entering/leaving fp32 matmul mode costs
    ~0.6us of PE pipeline each time (measured 212+432ns), so per-m
    interleaving pays it 8x per segment, the burst once.
  * Segments are (2,2,2,1,1) chunks wide: the last two are single-chunk
    so the final token chunk's gather+combine tail is half as deep, and
    the second-to-last chunk's tail overlaps the last segment's compute.
  * Combines pop one per slot while >=3 chunks are pending; output is
    written f16 (host upcasts) to halve the output DMA.

fp16x3 passes (hi*hi, lo_w*hi_x, hi_w*lo_x) are required: the min top-2/3
prob margin on this data is ~5e-6; fewer passes (or fp8 cross terms)
misrank tokens, and a single misranked token blows the absmax budget.
"""

import numpy as np

B, S, H, E = 4, 2048, 1024, 8
N_CORES = 8
T = (B * S) // N_CORES  # tokens per core
P = 128  # partitions
TCH = T // P  # token chunks per core (8)
KT = H // P  # contraction tiles (8)
HAL = 512  # psum pad width (full bank)
SEGS = [(0, 2), (2, 4), (4, 6), (6, 7), (7, 8)]
NSEG = len(SEGS)
SW = 2 * P  # max segment width (tokens)
CBLOB = KT + KT * E + E + 1 + 1  # b1 | w2 | ident | b2 | iota-bits = 82 cols

_compiled_nc = None


def _build():
    import concourse.bacc as bacc
    import concourse.bass as bass
    import concourse.tile as tile
    from concourse import mybir

    f32 = mybir.dt.float32
    f16 = mybir.dt.float16
    bf16 = mybir.dt.bfloat16
    u16 = mybir.dt.uint16
    u32 = mybir.dt.uint32
    nc = bacc.Bacc("TRN2", target_bir_lowering=False, debug=False,
                   num_devices=N_CORES)

    segw = [(c1 - c0) * P for c0, c1 in SEGS]
    # x: [p, k, hl, u] u16; seg0 additionally pair-split for early start
    xs0 = nc.dram_tensor("xs0", [KT // 2, P, 2, 2, SW], u16,
                         kind="ExternalInput").ap()
    xsd = [nc.dram_tensor(f"xs{s}", [P, KT, 2, segw[s]], u16,
                          kind="ExternalInput").ap()
           for s in range(1, NSEG)]
    # w1: k0 and k1 solo, then k-pairs: [p, (kk,) hl, m] u16
    w1k0 = nc.dram_tensor("w1k0", [P, 2, H], u16, kind="ExternalInput").ap()
    w1k1 = nc.dram_tensor("w1k1", [P, 2, H], u16, kind="ExternalInput").ap()
    w1p = nc.dram_tensor("w1p", [3, P, 2, 2, H], u16, kind="ExternalInput").ap()
    eo = nc.dram_tensor("eo", [E * T, H], f16, kind="ExternalInput").ap()
    cblob = nc.dram_tensor("cblob", [P, CBLOB], f32, kind="ExternalInput").ap()
    out = nc.dram_tensor("out", [T, H], f16, kind="ExternalOutput").ap()

    with tile.TileContext(nc) as tc:
        with (
            tc.tile_pool(name="singles", bufs=1) as singles,
            tc.tile_pool(name="eopool", bufs=6) as eopool,
            tc.tile_pool(name="accpool", bufs=4) as accpool,
            tc.tile_pool(name="smalls", bufs=8) as smalls,
            tc.tile_pool(name="ltpool", bufs=2) as ltpool,
            tc.tile_pool(name="psum", bufs=8, space="PSUM") as psum,
        ):
            # ---- input DMAs, criticality-ordered, ring-balanced ----
            x0_t = []
            for pr in range(2):  # first two k-pairs of seg0 before anything
                t = singles.tile([P, 2, 2, SW], u16, tag="x0", name=f"x0_{pr}",
                                 bufs=KT // 2)
                nc.sync.dma_start(out=t[:], in_=xs0[pr])
                x0_t.append(t)
            w1k0_t = singles.tile([P, 2, H], u16)
            nc.scalar.dma_start(out=w1k0_t[:], in_=w1k0)
            w1k1_t = singles.tile([P, 2, H], u16)
            nc.scalar.dma_start(out=w1k1_t[:], in_=w1k1)
            for pr in range(2, KT // 2):
                t = singles.tile([P, 2, 2, SW], u16, tag="x0", name=f"x0_{pr}",
                                 bufs=KT // 2)
                nc.sync.dma_start(out=t[:], in_=xs0[pr])
                x0_t.append(t)
            w1p_t = []
            for q in range(3):
                t = singles.tile([P, 2, 2, H], u16, tag="w1p", name=f"w1p{q}",
                                 bufs=3)
                nc.scalar.dma_start(out=t[:], in_=w1p[q])
                w1p_t.append(t)
            xs_t = {}
            for s, ring in ((1, nc.sync), (3, nc.sync), (4, nc.sync)):
                t = singles.tile([P, KT, 2, segw[s]], u16, tag=f"xs{s}",
                                 name=f"xs{s}")
                ring.dma_start(out=t[:], in_=xsd[s - 1])
                xs_t[s] = t
            cb = singles.tile([P, CBLOB], f32)
            nc.scalar.dma_start(out=cb[:], in_=cblob)
            for s, ring in ((2, nc.scalar),):
                t = singles.tile([P, KT, 2, segw[s]], u16, tag=f"xs{s}",
                                 name=f"xs{s}")
                ring.dma_start(out=t[:], in_=xsd[s - 1])
                xs_t[s] = t

            b1_sb = cb[:, 0:KT]                       # b1_sb[p,m] = b1[m*128+p]
            w2col = lambda j: cb[:, KT + j * E:KT + (j + 1) * E]  # W2[j*128+p, e]
            ident = cb[0:E, KT + KT * E:KT + KT * E + E]          # eye(8)
            b2_sb = cb[0:E, KT + KT * E + E:KT + KT * E + E + 1]
            iota_u = cb[:, CBLOB - 1:CBLOB].bitcast(u32)

            def xsl(si, k, hl):
                dt = f16 if hl == 0 else bf16
                if si == 0:
                    return x0_t[k // 2][:, k % 2, hl, :].bitcast(dt)
                return xs_t[si][:, k, hl, :].bitcast(dt)

            def wsl(k, hl, msl):
                dt = f16 if hl == 0 else bf16
                if k == 0:
                    return w1k0_t[:, hl, msl].bitcast(dt)
                if k == 1:
                    return w1k1_t[:, hl, msl].bitcast(dt)
                return w1p_t[(k - 2) // 2][:, (k - 2) % 2, hl, msl].bitcast(dt)

            hT = singles.tile([P, KT, T], f32)  # hT[p,m,t] = relu(x@W1+b1)[t, m*128+p]

            def mm3(ps, si, m, k, start, stop):
                msl = slice(m * P, (m + 1) * P)
                wh, wl = wsl(k, 0, msl), wsl(k, 1, msl)
                nc.tensor.matmul(ps[:], lhsT=wh, rhs=xsl(si, k, 0),
                                 start=start, stop=False)
                nc.tensor.matmul(ps[:], lhsT=wl, rhs=xsl(si, k, 0),
                                 start=False, stop=False)
                nc.tensor.matmul(ps[:], lhsT=wh, rhs=xsl(si, k, 1),
                                 start=False, stop=stop)

            # ---- deferred work queues ----
            # s3q: stage-3 (W2) matmuls for the finished segment, burst out
            # in the next segment's m=0 slot (one fp32 mode transition).
            # paq: per-chunk transpose+softmax+top2+gather-launch work.
            # pending: chunks whose gather is in flight, awaiting combine.
            s3q = []
            paq = []
            pending = []

            def emit_relu(ps, ps3, si, m):
                sl = slice(SEGS[si][0] * P, SEGS[si][1] * P)
                nc.scalar.activation(
                    out=hT[:, m, sl], in_=ps[:],
                    func=mybir.ActivationFunctionType.Relu,
                    bias=b1_sb[:, m:m + 1], scale=1.0)
                s3q.append((ps3, m, sl, si))

            def emit_stage3(ent):
                ps3, m, sl, si = ent
                nc.tensor.matmul(
                    ps3[:], lhsT=w2col(m), rhs=hT[:, m, sl],
                    start=(m == 0), stop=(m == KT - 1))
                if m == KT - 1:
                    c0, c1 = SEGS[si]
                    lT = ltpool.tile([E, segw[si]], f32, tag="lT", name="lT",
                                     padded_shape=[E, HAL])
                    nc.scalar.activation(
                        out=lT[:], in_=ps3[:],
                        func=mybir.ActivationFunctionType.Identity,
                        bias=b2_sb, scale=1.0)
                    for tch in range(c0, c1):
                        paq.append((lT, c0, tch))

            # phase A: transpose+softmax+top-2+gather launch for a 128-chunk
            def chunk_phase_a(lT, c0, tch):
                a = tch - c0
                pl = psum.tile([P, E], f32, tag="pl", name="pl", bufs=1,
                               padded_shape=[P, HAL])
                nc.tensor.transpose(pl[:], lT[:, a * P:(a + 1) * P], ident)
                negmax = smalls.tile([P, 1], f32, tag="negmax", name="negmax")
                nc.vector.reduce_max(negmax[:], pl[:],
                                     axis=mybir.AxisListType.X, negate=True)
                exps = smalls.tile([P, E], f32, tag="exps", name="exps")
                nc.scalar.activation(exps[:], pl[:],
                                     func=mybir.ActivationFunctionType.Exp,
                                     bias=negmax[:], scale=1.0)
                ssum = smalls.tile([P, 1], f32, tag="ssum", name="ssum")
                nc.vector.reduce_sum(ssum[:], exps[:], axis=mybir.AxisListType.X)
                rs = smalls.tile([P, 1], f32, tag="rs", name="rs")
                nc.vector.reciprocal(rs[:], ssum[:])
                # top-2 of exps == top-2 of probs; gate = exp * (1/sum)
                mx8 = smalls.tile([P, 8], f32, tag="mx8", name="mx8")
                nc.vector.max(mx8[:], exps[:])
                idx8 = smalls.tile([P, 8], u32, tag="idx8", name="idx8")
                nc.vector.max_index(idx8[:], mx8[:], exps[:])
                # flat eo row = expert*T + (tch*128 + partition)
                base = smalls.tile([P, 1], u32, tag="base", name="base")
                nc.vector.tensor_scalar_add(base[:], iota_u, tch * P)
                rows = smalls.tile([P, 2], u32, tag="rows", name="rows")
                for s in range(2):
                    nc.vector.tensor_scalar(
                        rows[:, s:s + 1], idx8[:, s:s + 1],
                        scalar1=T, scalar2=None, op0=mybir.AluOpType.mult)
                    nc.vector.tensor_tensor(
                        out=rows[:, s:s + 1], in0=rows[:, s:s + 1],
                        in1=base[:], op=mybir.AluOpType.add)
                eo_g = eopool.tile([P, 2, H], f16, tag="eog", name="eog")
                # NOTE: a single [P,2]-offset indirect DMA compiles and
                # simulates but dies at runtime (NRT INTERNAL) -- keep two
                # single-offset gathers
                for s in range(2):
                    nc.gpsimd.indirect_dma_start(
                        out=eo_g[:, s, :], out_offset=None, in_=eo,
                        in_offset=bass.IndirectOffsetOnAxis(
                            ap=rows[:, s:s + 1], axis=0))
                g0 = smalls.tile([P, 1], f32, tag="g0", name="g0")
                nc.vector.tensor_mul(g0[:], mx8[:, 0:1], rs[:])
                g1 = smalls.tile([P, 1], f32, tag="g1", name="g1")
                nc.vector.tensor_mul(g1[:], mx8[:, 1:2], rs[:])
                pending.append((tch, eo_g, g0, g1))

            # phase B: weighted combine + f16 output store
            def chunk_phase_b(st, flush=False):
                tch, eo_g, g0, g1 = st
                acc = accpool.tile([P, H], f16, tag="acc", name="acc")
                osl = slice(tch * P, (tch + 1) * P)
                if flush:
                    # tail: h-halves in parallel (scalar ACT || DVE mul),
                    # each half's output DMA as soon as it lands
                    h0, h1 = slice(0, H // 2), slice(H // 2, H)
                    nc.scalar.activation(acc[:, h0], eo_g[:, 0, h0],
                                         func=mybir.ActivationFunctionType.Copy,
                                         scale=g0[:])
                    nc.vector.tensor_scalar_mul(acc[:, h1], eo_g[:, 0, h1],
                                                g0[:])
                    for half, ring in ((h0, nc.sync), (h1, nc.scalar)):
                        nc.vector.scalar_tensor_tensor(
                            out=acc[:, half], in0=eo_g[:, 1, half],
                            scalar=g1[:], in1=acc[:, half],
                            op0=mybir.AluOpType.mult, op1=mybir.AluOpType.add)
                        ring.dma_start(out=out[osl, half], in_=acc[:, half])
                else:
                    nc.vector.tensor_scalar_mul(acc[:], eo_g[:, 0, :], g0[:])
                    nc.vector.scalar_tensor_tensor(
                        out=acc[:], in0=eo_g[:, 1, :], scalar=g1[:], in1=acc[:],
                        op0=mybir.AluOpType.mult, op1=mybir.AluOpType.add)
                    nc.sync.dma_start(out=out[osl, :], in_=acc[:])

            def slot(m):
                # one non-burst slot after a stage-2 block
                if m == 0:
                    while s3q:  # previous segment's stage3s, one fp32 burst
                        emit_stage3(s3q.pop(0))
                else:
                    if paq:
                        lT, c0, tch = paq.pop(0)
                        chunk_phase_a(lT, c0, tch)
                    if len(pending) >= 3:
                        chunk_phase_b(pending.pop(0))

            # ---- segment 0: k-outer over two m-halves (x and W1 k-slices
            # are consumed as the DMA delivers them) ----
            ps3_seg0 = None
            for half in range(2):
                ms = range(half * 4, half * 4 + 4)
                ps_m = {m: psum.tile([P, SW], f32, tag="ps2", name=f"ps{m}",
                                     bufs=5, padded_shape=[P, HAL])
                        for m in ms}
                for k in range(KT):
                    for m in ms:
                        mm3(ps_m[m], 0, m, k, k == 0, k == KT - 1)
                if half == 0:
                    ps3_seg0 = psum.tile([E, SW], f32, tag="ps3", name="ps3",
                                         bufs=2, padded_shape=[E, HAL])
                for m in ms:
                    emit_relu(ps_m[m], ps3_seg0, 0, m)

            # ---- segments 1+: m-outer, pipelined slots ----
            for si in range(1, NSEG):
                ps3 = psum.tile([E, segw[si]], f32, tag="ps3", name="ps3",
                                bufs=2, padded_shape=[E, HAL])
                for m in range(KT):
                    ps = psum.tile([P, segw[si]], f32, tag="ps2", name="ps",
                                   bufs=5, padded_shape=[P, HAL])
                    for k in range(KT):
                        mm3(ps, si, m, k, k == 0, k == KT - 1)
                    slot(m)
                    emit_relu(ps, ps3, si, m)

            # ---- tail: last segment's stage3 burst, phase_a, flush ----
            while s3q:
                emit_stage3(s3q.pop(0))
            while paq:
                lT, c0, tch = paq.pop(0)
                chunk_phase_a(lT, c0, tch)
            while pending:
                chunk_phase_b(pending.pop(0), flush=True)

    nc.compile()
    return nc


def _get_nc():
    global _compiled_nc
    if _compiled_nc is None:
        _compiled_nc = _build()
    return _compiled_nc


def _split_hi_lo(a):
    """fp16 hi + bf16 lo split of an fp32 array (lo unscaled; bf16's
    exponent range covers it)."""
    import ml_dtypes
    a = np.asarray(a, dtype=np.float32)
    hi = a.astype(np.float16)
    lo = (a.astype(np.float64) - hi.astype(np.float64)).astype(ml_dtypes.bfloat16)
    return hi, lo


def make_in_maps(hidden_states, expert_outputs, W1, b1, W2, b2):
    hs = np.ascontiguousarray(np.asarray(hidden_states, dtype=np.float32)).reshape(B * S, H)
    eo = np.asarray(expert_outputs, dtype=np.float32).reshape(E, B * S, H)
    w1hi, w1lo = _split_hi_lo(W1)
    # w1 blob [k, p, hl, m] u16 = bits of W1[(k*128+p), m] hi/lo
    w1u = np.empty((KT, P, 2, H), dtype=np.uint16)
    w1u[:, :, 0, :] = w1hi.reshape(KT, P, H).view(np.uint16)
    w1u[:, :, 1, :] = w1lo.reshape(KT, P, H).view(np.uint16)
    w1k0 = np.ascontiguousarray(w1u[0])
    w1k1 = np.ascontiguousarray(w1u[1])
    # pairs [q, p, kk, hl, m]
    w1pr = np.ascontiguousarray(w1u[2:].reshape(3, 2, P, 2, H).transpose(0, 2, 1, 3, 4))
    b1v = np.asarray(b1, dtype=np.float32)
    w2 = np.asarray(W2, dtype=np.float32)
    b2v = np.asarray(b2, dtype=np.float32)
    # constants blob: b1 | w2 | ident | b2 | iota-bits
    cblk = np.zeros((P, CBLOB), dtype=np.float32)
    cblk[:, 0:KT] = b1v.reshape(KT, P).T                    # b1[m*128+p]
    cblk[:, KT:KT + KT * E] = w2.reshape(KT, P, E).transpose(1, 0, 2).reshape(P, KT * E)
    cblk[0:E, KT + KT * E:KT + KT * E + E] = np.eye(E, dtype=np.float32)
    cblk[0:E, KT + KT * E + E] = b2v
    cblk[:, CBLOB - 1] = np.arange(P, dtype=np.uint32).view(np.float32)
    in_maps = []
    for c in range(N_CORES):
        sl = slice(c * T, (c + 1) * T)
        xhi, xlo = _split_hi_lo(hs[sl].T)  # [H, T]

        def blob(c0, c1):
            w = (c1 - c0) * P
            u = np.empty((P, KT, 2, w), dtype=np.uint16)
            u[:, :, 0, :] = (xhi[:, c0 * P:c1 * P].reshape(KT, P, w)
                             .transpose(1, 0, 2).view(np.uint16))
            u[:, :, 1, :] = (xlo[:, c0 * P:c1 * P].reshape(KT, P, w)
                             .transpose(1, 0, 2).view(np.uint16))
            return u

        # seg0 pair-major: [pr, p, kk, hl, u]
        b0 = blob(*SEGS[0])
        xs0 = np.ascontiguousarray(
            b0.reshape(P, KT // 2, 2, 2, SW).transpose(1, 0, 2, 3, 4))
        m = {"xs0": xs0, "w1k0": w1k0, "w1k1": w1k1, "w1p": w1pr,
             "cblob": cblk,
             "eo": np.ascontiguousarray(
                 eo[:, sl, :].reshape(E * T, H).astype(np.float16))}
        for s in range(1, NSEG):
            m[f"xs{s}"] = np.ascontiguousarray(blob(*SEGS[s]))
        in_maps.append(m)
    return in_maps


def kernel(hidden_states, expert_outputs, W1, b1, W2, b2, k=2):
    from concourse.bass_utils import run_bass_kernel_spmd

    in_maps = make_in_maps(hidden_states, expert_outputs, W1, b1, W2, b2)
    nc = _get_nc()
    res = run_bass_kernel_spmd(nc, in_maps, core_ids=list(range(N_CORES)))
    full = np.concatenate([res.results[c]["out"].astype(np.float32)
                           for c in range(N_CORES)], axis=0)
    return full.reshape(B, S, H)
